# revision 1
# baseline (speedup 1.0000x reference)
"""AugmentedLSTMCell on 8 TRN2 NeuronCores — data-parallel over batch.

Layout: feature-on-partition (transposed). Per core: B_loc=2048 batch rows.
  proj.T[j, b] = sum_e W[j, e] * in[b, e]
  lhsT tiles  = W.T blocks [128e, 128j]  (host pre-packed, bf16)
  rhs         = in.T        [128e, 2048b] (host pre-transposed, bf16)
  psum [128j, 2048b] accumulates 8 k-tiles of Wi-proj + 8 k-tiles of Ws-proj
  (the "fused = proj_in + proj_st" add comes free via PSUM accumulation).
  ScalarE applies per-feature bias + sigmoid/tanh straight out of PSUM.
Host transposes outputs back to [B, H].
"""
import sys
import types

sys.path.insert(0, "/opt/trn_rl_repo")
sys.path.insert(0, "/root/.axon_site")

# Shim antenv.axon_hooks (missing on this image) so trace=True can profile.
if "antenv.axon_hooks" not in sys.modules:
    _hooks = types.ModuleType("antenv.axon_hooks")
    _state = {"hook": None}
    _hooks.set_axon_ntff_profile_hook = lambda h: _state.__setitem__("hook", h)
    _hooks.get_axon_ntff_profile_hook = lambda: _state["hook"]
    sys.modules["antenv.axon_hooks"] = _hooks
    try:
        from trn_agent_boot.trn_boot import _ntff_profile_via_ctypes

        _hooks.set_axon_ntff_profile_hook(
            _ntff_profile_via_ctypes("/opt/axon/libaxon_pjrt.so")
        )
    except Exception:
        pass

import numpy as np
import ml_dtypes

import concourse.bass as bass
import concourse.bacc as bacc
import concourse.mybir as mybir
from concourse import tile
from concourse.bass_utils import run_bass_kernel_spmd

BF16 = ml_dtypes.bfloat16

N_CORES = 8
B, E, H = 16384, 1024, 1024
BL = B // N_CORES          # 2048 batch rows per core
KT = E // 128              # 8 contraction k-tiles
NJI = 6 * H // 128         # 48 feature tiles of proj_in
NJS = 5 * H // 128         # 40 feature tiles of proj_st
NT = H // 128              # 8 H-slices
BC = 512                   # matmul moving free dim (one PSUM bank)
NBC = BL // BC             # batch chunks per matmul group

AF = mybir.ActivationFunctionType


def build_nc():
    nc = bacc.Bacc(None, target_bir_lowering=False)
    f32, bf16 = mybir.dt.float32, mybir.dt.bfloat16

    xT = nc.declare_dram_parameter("xT", [E, BL], bf16, isOutput=False)
    hT = nc.declare_dram_parameter("hT", [H, BL], bf16, isOutput=False)
    cT = nc.declare_dram_parameter("cT", [H, BL], bf16, isOutput=False)
    wi = nc.declare_dram_parameter("wi", [NJI, 128, E], bf16, isOutput=False)
    ws = nc.declare_dram_parameter("ws", [NJS, 128, H], bf16, isOutput=False)
    bias = nc.declare_dram_parameter("bias", [128, NJI], f32, isOutput=False)
    outT = nc.declare_dram_parameter("outT", [H, BL], f32, isOutput=True)
    memT = nc.declare_dram_parameter("memT", [H, BL], f32, isOutput=True)

    with tile.TileContext(nc) as tc:
        with (
            tc.tile_pool(name="resident", bufs=1) as resident,
            tc.tile_pool(name="wpool", bufs=8) as wpool,
            tc.tile_pool(name="cpool", bufs=2) as cpool,
            tc.tile_pool(name="psum", bufs=2, space="PSUM") as psum_pool,
            tc.tile_pool(name="gates", bufs=10) as gate_pool,
            tc.tile_pool(name="tmp", bufs=6) as tmp_pool,
            tc.tile_pool(name="outp", bufs=2) as out_pool,
        ):
            # Resident: full xT/hT (reused by every feature tile) + bias.
            # Startup is DMA-queue-bound (~20 GB/s per queue), so load in
            # strict need-order, split into ~128KB pieces spread over queues:
            # xt0 + first weight tile first, then the rest.
            def split_dma(dst, src, nsplit, eng=None):
                eng = eng or nc.sync
                n = dst.shape[-1]
                per = n // nsplit
                for q in range(nsplit):
                    sl = slice(q * per, (q + 1) * per)
                    eng.dma_start(dst[:, sl], src[:, sl])

            bias_sb = resident.tile([128, NJI], f32, tag="bias")
            nc.sync.dma_start(bias_sb[:], bias[:])

            xt_k = [None] + [
                resident.tile([128, BL], bf16, tag=f"xt{k}", name=f"xt{k}")
                for k in range(1, KT)
            ]
            # k=0 is split into two half-tiles so the very first matmuls
            # (bc 0-1) depend on only 256KB of x instead of the full 512KB.
            xt0a = resident.tile([128, BL // 2], bf16, tag="xt0a", name="xt0a")
            xt0b = resident.tile([128, BL // 2], bf16, tag="xt0b", name="xt0b")

            def rhs_x(k, bc):
                if k == 0:
                    t = xt0a if bc < 2 else xt0b
                    return t[:, (bc % 2) * BC : (bc % 2 + 1) * BC]
                return xt_k[k][:, bc * BC : (bc + 1) * BC]
            ht_k = [
                resident.tile([128, BL], bf16, tag=f"ht{k}", name=f"ht{k}")
                for k in range(KT)
            ]
            # Preloaded weight tiles for the first feature tiles: two hw_proj
            # tiles (x-only — PE works on these while h streams in) and the
            # first i-gate tile.
            w_hwp0 = wpool.tile([128, E], bf16, tag="w")
            w_hwp1 = wpool.tile([128, E], bf16, tag="w")
            w_hwp2 = wpool.tile([128, E], bf16, tag="w")
            w_i0 = wpool.tile([128, E], bf16, tag="w")
            w_s0 = wpool.tile([128, H], bf16, tag="w")
            split_dma(xt0a, xT[0:128, : BL // 2], 2, eng=nc.gpsimd)
            split_dma(w_hwp0, wi[5 * NT + 0], 2, eng=nc.gpsimd)
            split_dma(xt0b, xT[0:128, BL // 2 :], 2, eng=nc.gpsimd)
            split_dma(xt_k[1], xT[128:256, :], 4, eng=nc.gpsimd)
            for k in range(2, 3):
                split_dma(xt_k[k], xT[k * 128 : (k + 1) * 128, :], 4)
            split_dma(w_hwp1, wi[5 * NT + 1], 2)
            split_dma(w_hwp2, wi[5 * NT + 2], 2)
            split_dma(w_i0, wi[0], 2)
            split_dma(w_s0, ws[0], 2)
            for k in range(3, KT):
                split_dma(xt_k[k], xT[k * 128 : (k + 1) * 128, :], 4)
            for k in range(KT):
                split_dma(ht_k[k], hT[k * 128 : (k + 1) * 128, :], 4)

            def feature_tile(jt, func, w_i=None, w_s=None, chunk_act=1,
                             bc0=0, bc1=NBC):
                """proj tile [128j, (bc1-bc0)*BC] -> activated gate (bf16)."""
                if w_i is None:
                    w_i = wpool.tile([128, E], bf16, tag="w")
                    nc.sync.dma_start(w_i[:], wi[jt])
                has_st = jt < NJS
                if has_st and w_s is None:
                    w_s = wpool.tile([128, H], bf16, tag="w")
                    nc.sync.dma_start(w_s[:], ws[jt])
                width = (bc1 - bc0) * BC
                ps = psum_pool.tile([128, width], f32, tag="ps")
                for k in range(KT):
                    lhsT = w_i[:, k * 128 : (k + 1) * 128]
                    for bc in range(bc0, bc1):
                        lo = (bc - bc0) * BC
                        nc.tensor.matmul(
                            ps[:, lo : lo + BC],
                            lhsT,
                            rhs_x(k, bc),
                            start=(k == 0),
                            stop=(not has_st and k == KT - 1),
                        )
                if has_st:
                    for k in range(KT):
                        lhsT = w_s[:, k * 128 : (k + 1) * 128]
                        for bc in range(bc0, bc1):
                            lo = (bc - bc0) * BC
                            nc.tensor.matmul(
                                ps[:, lo : lo + BC],
                                lhsT,
                                ht_k[k][:, bc * BC : (bc + 1) * BC],
                                start=False,
                                stop=(k == KT - 1),
                            )
                g = gate_pool.tile([128, width], bf16, tag="g")
                cw = width // chunk_act
                for a in range(chunk_act):
                    sl = slice(a * cw, (a + 1) * cw)
                    nc.scalar.activation(
                        g[:, sl], ps[:, sl], func, bias=bias_sb[:, jt : jt + 1]
                    )
                return g

            mult, addop, subop = (
                mybir.AluOpType.mult,
                mybir.AluOpType.add,
                mybir.AluOpType.subtract,
            )

            hwp_pre = [
                feature_tile(5 * NT + 0, AF.Identity, w_i=w_hwp0),
                feature_tile(5 * NT + 1, AF.Identity, w_i=w_hwp1),
                feature_tile(5 * NT + 2, AF.Identity, w_i=w_hwp2),
            ]
            for t in range(NT):
                hwp = (
                    hwp_pre[t]
                    if t < len(hwp_pre)
                    else feature_tile(5 * NT + t, AF.Identity)
                )
                i_g = feature_tile(
                    t, AF.Sigmoid, w_i=w_i0 if t == 0 else None,
                    w_s=w_s0 if t == 0 else None,
                )
                m_g = feature_tile(2 * NT + t, AF.Tanh)
                f_g = feature_tile(NT + t, AF.Sigmoid)
                o_g = feature_tile(3 * NT + t, AF.Sigmoid)

                ct = cpool.tile([128, BL], bf16, tag="c")
                nc.sync.dma_start(ct[:], cT[t * 128 : (t + 1) * 128, :])

                t1 = tmp_pool.tile([128, BL], bf16, tag="tmp")
                nc.vector.tensor_tensor(t1[:], i_g[:], m_g[:], mult)
                t2 = tmp_pool.tile([128, BL], bf16, tag="tmp")
                nc.vector.tensor_tensor(t2[:], f_g[:], ct[:], mult)
                mem = out_pool.tile([128, BL], f32, tag="mem")
                nc.vector.tensor_tensor(mem[:], t1[:], t2[:], addop)
                nc.sync.dma_start(memT[t * 128 : (t + 1) * 128, :], mem[:])

                tmem = tmp_pool.tile([128, BL], bf16, tag="tmp")
                nc.scalar.activation(tmem[:], mem[:], AF.Tanh)
                outp = tmp_pool.tile([128, BL], bf16, tag="tmp")
                nc.vector.tensor_tensor(outp[:], o_g[:], tmem[:], mult)
                # out = hwp + hw*(outp - hwp), chunked so the tail after the
                # final hw matmuls pipelines with the output DMA.
                u = tmp_pool.tile([128, BL], bf16, tag="tmp")
                nc.vector.tensor_tensor(u[:], outp[:], hwp[:], subop)

                def blend(hw_tile, col0, ncols, nchunk):
                    # out[:, col0:col0+ncols] = hwp + hw*u over `nchunk` pieces
                    ec = ncols // nchunk
                    for e in range(nchunk):
                        sl = slice(col0 + e * ec, col0 + (e + 1) * ec)
                        lsl = slice(e * ec, (e + 1) * ec)
                        v = tmp_pool.tile([128, ec], bf16, tag="v")
                        nc.vector.tensor_tensor(v[:], hw_tile[:, lsl], u[:, sl], mult)
                        outf = out_pool.tile([128, ec], f32, tag="out")
                        nc.vector.tensor_tensor(outf[:], v[:], hwp[:, sl], addop)
                        nc.sync.dma_start(outT[t * 128 : (t + 1) * 128, sl], outf[:])

                if t < NT - 1:
                    hw_g = feature_tile(4 * NT + t, AF.Sigmoid, chunk_act=4)
                    blend(hw_g, 0, BL, 4)
                else:
                    # Last group: split the hw tile in half so the first
                    # half's blend+DMA overlaps the second half's matmuls.
                    for half in range(2):
                        hw_h = feature_tile(
                            4 * NT + t, AF.Sigmoid, chunk_act=2,
                            bc0=2 * half, bc1=2 * half + 2,
                        )
                        blend(hw_h, half * (BL // 2), BL // 2, 2)

    nc.compile()
    return nc


_NC_CACHE = None


def _get_nc():
    global _NC_CACHE
    if _NC_CACHE is None:
        _NC_CACHE = build_nc()
    return _NC_CACHE


def _pack_weights(W, njt):
    # W [njt*128 j, K e] -> [njt, 128 p, K] with [jt, p, k*128+m] = W[jt*128+m, k*128+p]
    K = W.shape[1]
    kt = K // 128
    return np.ascontiguousarray(
        W.reshape(njt, 128, kt, 128).transpose(0, 3, 2, 1).reshape(njt, 128, K)
    ).astype(BF16)


def prepare_in_maps(x, h, c, Wi, bi, Ws, bs):
    wi_p = _pack_weights(np.asarray(Wi, np.float32), NJI)
    ws_p = _pack_weights(np.asarray(Ws, np.float32), NJS)
    bias_comb = np.concatenate(
        [np.asarray(bi[: 5 * H], np.float32) + np.asarray(bs, np.float32),
         np.asarray(bi[5 * H :], np.float32)]
    )
    bias_pack = np.ascontiguousarray(bias_comb.reshape(NJI, 128).T).astype(np.float32)

    in_maps = []
    for i in range(N_CORES):
        s = slice(i * BL, (i + 1) * BL)
        in_maps.append(
            {
                "xT": np.ascontiguousarray(np.asarray(x[s], np.float32).T).astype(BF16),
                "hT": np.ascontiguousarray(np.asarray(h[s], np.float32).T).astype(BF16),
                "cT": np.ascontiguousarray(np.asarray(c[s], np.float32).T).astype(BF16),
                "wi": wi_p,
                "ws": ws_p,
                "bias": bias_pack,
            }
        )
    return in_maps


def run(in_maps, trace=False):
    nc = _get_nc()
    res = run_bass_kernel_spmd(nc, in_maps, core_ids=list(range(N_CORES)), trace=trace)
    out = np.empty((B, H), np.float32)
    mem = np.empty((B, H), np.float32)
    for i in range(N_CORES):
        s = slice(i * BL, (i + 1) * BL)
        out[s] = res.results[i]["outT"].T
        mem[s] = res.results[i]["memT"].T
    return (out, mem), res


def kernel(x, h, c, Wi, bi, Ws, bs):
    in_maps = prepare_in_maps(x, h, c, Wi, bi, Ws, bs)
    (out, mem), _ = run(in_maps, trace=False)
    return out, mem



# revision 4
# speedup vs baseline: 1.4308x; 1.4308x over previous
"""AugmentedLSTMCell on 8 TRN2 NeuronCores — data-parallel over batch.

Layout: feature-on-partition (transposed). Per core: B_loc=2048 batch rows.
  proj.T[j, b] = sum_e W[j, e] * in[b, e]
  psum [128j, 2048b] accumulates Wi-proj + Ws-proj k-tiles
  (the "fused = proj_in + proj_st" add comes free via PSUM accumulation).
  ScalarE applies per-feature bias + sigmoid/tanh straight out of PSUM.

Mixed precision per gate block (i,f,m,o,hw,hwp):
  - sigmoid gates (i,f,o,hw): fp8-e4m3 weights+activations with
    perf_mode=DoubleRow (256-deep contraction per matmul, ~1.7x PE rate).
    Weights pre-scaled x64 to clear the e4m3 subnormal range; compensated
    by scale=1/64 in the ScalarE activation.
  - tanh block (m) and the linear passthrough (hwp): bf16 (error-critical
    paths; fp8 there pushes rel_err past the 2e-2 gate).
Host transposes outputs back to [B, H].
"""
import sys
import types

sys.path.insert(0, "/opt/trn_rl_repo")
sys.path.insert(0, "/root/.axon_site")

# Shim antenv.axon_hooks (missing on this image) so trace=True can profile.
if "antenv.axon_hooks" not in sys.modules:
    _hooks = types.ModuleType("antenv.axon_hooks")
    _state = {"hook": None}
    _hooks.set_axon_ntff_profile_hook = lambda h: _state.__setitem__("hook", h)
    _hooks.get_axon_ntff_profile_hook = lambda: _state["hook"]
    sys.modules["antenv.axon_hooks"] = _hooks
    try:
        from trn_agent_boot.trn_boot import _ntff_profile_via_ctypes

        _hooks.set_axon_ntff_profile_hook(
            _ntff_profile_via_ctypes("/opt/axon/libaxon_pjrt.so")
        )
    except Exception:
        pass

import numpy as np
import ml_dtypes

import concourse.bass as bass
import concourse.bacc as bacc
import concourse.mybir as mybir
from concourse import tile
from concourse.bass_utils import run_bass_kernel_spmd

BF16 = ml_dtypes.bfloat16
F8 = ml_dtypes.float8_e4m3fn

N_CORES = 8
B, E, H = 16384, 1024, 1024
BL = B // N_CORES          # 2048 batch rows per core
KT = E // 128              # 8 contraction k-tiles
KP = KT // 2               # 4 DoubleRow k-pairs
NJI = 6 * H // 128         # 48 feature tiles of proj_in
NJS = 5 * H // 128         # 40 feature tiles of proj_st
NT = H // 128              # 8 H-slices
BC = 512                   # matmul moving free dim (one PSUM bank)
NBC = BL // BC             # batch chunks per matmul group
WS = 64.0                  # weight pre-scale (all blocks; undone in act)

# Per-block dtype: '8' = fp8 DoubleRow, 'b' = bf16.
# Blocks: 0=i, 1=f, 2=m(tanh), 3=o, 4=hw, 5=hw_proj(linear)
CFG_WI = ["8", "8", "b", "8", "8", "b"]
CFG_WS = ["8", "8", "b", "8", "8"]

AF = mybir.ActivationFunctionType
DR = mybir.MatmulPerfMode.DoubleRow


def build_nc():
    nc = bacc.Bacc(None, target_bir_lowering=False)
    f32, bf16, f8 = mybir.dt.float32, mybir.dt.bfloat16, mybir.dt.float8e4

    xb_d = nc.declare_dram_parameter("xb", [E, BL], bf16, isOutput=False)
    hb_d = nc.declare_dram_parameter("hb", [H, BL], bf16, isOutput=False)
    x8_d = nc.declare_dram_parameter("x8", [128, KT, BL], f8, isOutput=False)
    h8_d = nc.declare_dram_parameter("h8", [128, KT, BL], f8, isOutput=False)
    cT = nc.declare_dram_parameter("cT", [H, BL], bf16, isOutput=False)
    wib = nc.declare_dram_parameter("wib", [NJI, 128, E], bf16, isOutput=False)
    wsb = nc.declare_dram_parameter("wsb", [NJS, 128, H], bf16, isOutput=False)
    wi8 = nc.declare_dram_parameter("wi8", [NJI, 128, KT, 128], f8, isOutput=False)
    ws8 = nc.declare_dram_parameter("ws8", [NJS, 128, KT, 128], f8, isOutput=False)
    bias = nc.declare_dram_parameter("bias", [128, NJI], f32, isOutput=False)
    outT = nc.declare_dram_parameter("outT", [H, BL], bf16, isOutput=True)
    memT = nc.declare_dram_parameter("memT", [H, BL], bf16, isOutput=True)

    with tile.TileContext(nc) as tc:
        with (
            tc.tile_pool(name="resident", bufs=1) as resident,
            tc.tile_pool(name="wpool8", bufs=8) as wpool8,
            tc.tile_pool(name="wpoolb", bufs=4) as wpoolb,
            tc.tile_pool(name="cpool", bufs=2) as cpool,
            tc.tile_pool(name="psum", bufs=2, space="PSUM") as psum_pool,
            tc.tile_pool(name="gates", bufs=8) as gate_pool,
            tc.tile_pool(name="tmp", bufs=6) as tmp_pool,
            tc.tile_pool(name="outp", bufs=2) as out_pool,
        ):
            # ---- resident tiles -------------------------------------------
            bias_sb = resident.tile([128, NJI], f32, tag="bias")

            # bf16 x (for bf16 blocks): k=0 split in halves so the first
            # matmuls depend on only 256KB of x.
            xt_k = [None] + [
                resident.tile([128, BL], bf16, tag=f"xt{k}", name=f"xt{k}")
                for k in range(1, KT)
            ]
            xt0a = resident.tile([128, BL // 2], bf16, tag="xt0a", name="xt0a")
            xt0b = resident.tile([128, BL // 2], bf16, tag="xt0b", name="xt0b")

            def rhs_x(k, bc):
                if k == 0:
                    t = xt0a if bc < 2 else xt0b
                    return t[:, (bc % 2) * BC : (bc % 2 + 1) * BC]
                return xt_k[k][:, bc * BC : (bc + 1) * BC]

            ht_k = [
                resident.tile([128, BL], bf16, tag=f"ht{k}", name=f"ht{k}")
                for k in range(KT)
            ]
            # fp8 x/h, [128p, KT, BL] so a [:, 2kp:2kp+2, cols] slice is the
            # 3D DoubleRow rhs AP.
            x8s = resident.tile([128, KT, BL], f8, tag="x8", name="x8")
            h8s = resident.tile([128, KT, BL], f8, tag="h8", name="h8")

            def split_dma(dst, src, nsplit, eng=None):
                eng = eng or nc.sync
                n = dst.shape[-1]
                per = n // nsplit
                for q in range(nsplit):
                    sl = slice(q * per, (q + 1) * per)
                    eng.dma_start(dst[..., sl], src[..., sl])

            # ---- startup DMA in strict need-order -------------------------
            # t=0 tile order: hwp(bf16, x), i(fp8 x8/h8), f, o, m(bf16 x+h), hw
            w_hwp0 = wpoolb.tile([128, E], bf16, tag="wb")
            w_i0 = wpool8.tile([128, KT, 128], f8, tag="w8")
            w_s0 = wpool8.tile([128, KT, 128], f8, tag="w8")
            split_dma(xt0a, xb_d[0:128, : BL // 2], 2, eng=nc.gpsimd)
            split_dma(w_hwp0, wib[5 * NT + 0], 2, eng=nc.gpsimd)
            split_dma(xt0b, xb_d[0:128, BL // 2 :], 2, eng=nc.gpsimd)
            split_dma(xt_k[1], xb_d[128:256, :], 4, eng=nc.gpsimd)
            split_dma(xt_k[2], xb_d[256:384, :], 4, eng=nc.gpsimd)
            # fp8 resident + first fp8 weights (for i-tile), kp-ordered
            nc.scalar.dma_start(w_i0[:], wi8[0])
            for kp in range(KP):
                split_dma(x8s[:, 2 * kp : 2 * kp + 2, :], x8_d[:, 2 * kp : 2 * kp + 2, :], 2, eng=nc.scalar)
            nc.scalar.dma_start(w_s0[:], ws8[0])
            for kp in range(KP):
                split_dma(h8s[:, 2 * kp : 2 * kp + 2, :], h8_d[:, 2 * kp : 2 * kp + 2, :], 2, eng=nc.scalar)
            for k in range(3, KT):
                split_dma(xt_k[k], xb_d[k * 128 : (k + 1) * 128, :], 4, eng=nc.gpsimd)
            for k in range(KT):
                split_dma(ht_k[k], hb_d[k * 128 : (k + 1) * 128, :], 4, eng=nc.sync)
            nc.sync.dma_start(bias_sb[:], bias[:])

            # ---- one gate feature-tile ------------------------------------
            def feature_tile(jt, func, dt8, w_i=None, w_s=None, chunk_act=1,
                             bc0=0, bc1=NBC):
                """proj tile [128j, (bc1-bc0)*BC] -> activated gate (bf16)."""
                has_st = jt < NJS
                if dt8:
                    if w_i is None:
                        w_i = wpool8.tile([128, KT, 128], f8, tag="w8")
                        nc.sync.dma_start(w_i[:], wi8[jt])
                    if has_st and w_s is None:
                        w_s = wpool8.tile([128, KT, 128], f8, tag="w8")
                        nc.sync.dma_start(w_s[:], ws8[jt])
                else:
                    if w_i is None:
                        w_i = wpoolb.tile([128, E], bf16, tag="wb")
                        nc.sync.dma_start(w_i[:], wib[jt])
                    if has_st and w_s is None:
                        w_s = wpoolb.tile([128, H], bf16, tag="wb")
                        nc.sync.dma_start(w_s[:], wsb[jt])
                width = (bc1 - bc0) * BC
                ps = psum_pool.tile([128, width], f32, tag="ps")
                if dt8:
                    for kp in range(KP):
                        lhsT = w_i[:, 2 * kp : 2 * kp + 2, :]
                        for bc in range(bc0, bc1):
                            lo = (bc - bc0) * BC
                            nc.tensor.matmul(
                                ps[:, lo : lo + BC],
                                lhsT,
                                x8s[:, 2 * kp : 2 * kp + 2, bc * BC : (bc + 1) * BC],
                                start=(kp == 0),
                                stop=(not has_st and kp == KP - 1),
                                perf_mode=DR,
                            )
                    if has_st:
                        for kp in range(KP):
                            lhsT = w_s[:, 2 * kp : 2 * kp + 2, :]
                            for bc in range(bc0, bc1):
                                lo = (bc - bc0) * BC
                                nc.tensor.matmul(
                                    ps[:, lo : lo + BC],
                                    lhsT,
                                    h8s[:, 2 * kp : 2 * kp + 2, bc * BC : (bc + 1) * BC],
                                    start=False,
                                    stop=(kp == KP - 1),
                                    perf_mode=DR,
                                )
                else:
                    for k in range(KT):
                        lhsT = w_i[:, k * 128 : (k + 1) * 128]
                        for bc in range(bc0, bc1):
                            lo = (bc - bc0) * BC
                            nc.tensor.matmul(
                                ps[:, lo : lo + BC],
                                lhsT,
                                rhs_x(k, bc),
                                start=(k == 0),
                                stop=(not has_st and k == KT - 1),
                            )
                    if has_st:
                        for k in range(KT):
                            lhsT = w_s[:, k * 128 : (k + 1) * 128]
                            for bc in range(bc0, bc1):
                                lo = (bc - bc0) * BC
                                nc.tensor.matmul(
                                    ps[:, lo : lo + BC],
                                    lhsT,
                                    ht_k[k][:, bc * BC : (bc + 1) * BC],
                                    start=False,
                                    stop=(k == KT - 1),
                                )
                g = gate_pool.tile([128, width], bf16, tag="g")
                cw = width // chunk_act
                for a in range(chunk_act):
                    sl = slice(a * cw, (a + 1) * cw)
                    nc.scalar.activation(
                        g[:, sl], ps[:, sl], func,
                        bias=bias_sb[:, jt : jt + 1], scale=1.0 / WS,
                    )
                return g

            mult, addop, subop = (
                mybir.AluOpType.mult,
                mybir.AluOpType.add,
                mybir.AluOpType.subtract,
            )

            for t in range(NT):
                hwp = feature_tile(
                    5 * NT + t, AF.Identity, CFG_WI[5] == "8",
                    w_i=w_hwp0 if t == 0 else None,
                )
                i_g = feature_tile(
                    t, AF.Sigmoid, CFG_WI[0] == "8",
                    w_i=w_i0 if t == 0 else None,
                    w_s=w_s0 if t == 0 else None,
                )
                f_g = feature_tile(NT + t, AF.Sigmoid, CFG_WI[1] == "8")
                o_g = feature_tile(3 * NT + t, AF.Sigmoid, CFG_WI[3] == "8")
                m_g = feature_tile(2 * NT + t, AF.Tanh, CFG_WI[2] == "8")

                ct = cpool.tile([128, BL], bf16, tag="c")
                nc.sync.dma_start(ct[:], cT[t * 128 : (t + 1) * 128, :])

                t1 = tmp_pool.tile([128, BL], bf16, tag="tmp")
                nc.vector.tensor_tensor(t1[:], i_g[:], m_g[:], mult)
                t2 = tmp_pool.tile([128, BL], bf16, tag="tmp")
                nc.vector.tensor_tensor(t2[:], f_g[:], ct[:], mult)
                mem = out_pool.tile([128, BL], bf16, tag="mem")
                nc.vector.tensor_tensor(mem[:], t1[:], t2[:], addop)
                nc.sync.dma_start(memT[t * 128 : (t + 1) * 128, :], mem[:])

                tmem = tmp_pool.tile([128, BL], bf16, tag="tmp")
                nc.scalar.activation(tmem[:], mem[:], AF.Tanh)
                outp = tmp_pool.tile([128, BL], bf16, tag="tmp")
                nc.vector.tensor_tensor(outp[:], o_g[:], tmem[:], mult)
                # out = hwp + hw*(outp - hwp), chunked so the tail after the
                # final hw matmuls pipelines with the output DMA.
                u = tmp_pool.tile([128, BL], bf16, tag="tmp")
                nc.vector.tensor_tensor(u[:], outp[:], hwp[:], subop)

                def blend(hw_tile, col0, ncols, nchunk):
                    # out[:, col0:col0+ncols] = hwp + hw*u over `nchunk` pieces
                    ec = ncols // nchunk
                    for e in range(nchunk):
                        sl = slice(col0 + e * ec, col0 + (e + 1) * ec)
                        lsl = slice(e * ec, (e + 1) * ec)
                        v = tmp_pool.tile([128, ec], bf16, tag="v")
                        nc.vector.tensor_tensor(v[:], hw_tile[:, lsl], u[:, sl], mult)
                        outf = out_pool.tile([128, ec], bf16, tag="out")
                        nc.vector.tensor_tensor(outf[:], v[:], hwp[:, sl], addop)
                        nc.sync.dma_start(outT[t * 128 : (t + 1) * 128, sl], outf[:])

                if t < NT - 1:
                    hw_g = feature_tile(4 * NT + t, AF.Sigmoid, CFG_WI[4] == "8",
                                        chunk_act=4)
                    blend(hw_g, 0, BL, 4)
                else:
                    # Last group: split the hw tile in half so the first
                    # half's blend+DMA overlaps the second half's matmuls.
                    for half in range(2):
                        hw_h = feature_tile(
                            4 * NT + t, AF.Sigmoid, CFG_WI[4] == "8",
                            chunk_act=2, bc0=2 * half, bc1=2 * half + 2,
                        )
                        blend(hw_h, half * (BL // 2), BL // 2, 2)

    nc.compile()
    return nc


_NC_CACHE = None


def _get_nc():
    global _NC_CACHE
    if _NC_CACHE is None:
        _NC_CACHE = build_nc()
    return _NC_CACHE


def _pack_weights_bf16(W, njt):
    # W [njt*128 j, K e] -> [njt, 128 p, K] with [jt, p, k*128+m] = W[jt*128+m, k*128+p]
    K = W.shape[1]
    kt = K // 128
    return np.ascontiguousarray(
        (W * WS).reshape(njt, 128, kt, 128).transpose(0, 3, 2, 1).reshape(njt, 128, K)
    ).astype(BF16)


def _pack_weights_f8(W, njt):
    # W [njt*128 j, K e] -> [njt, 128 p, kt, 128 m] = W[jt*128+m, k*128+p]*WS
    K = W.shape[1]
    kt = K // 128
    return np.ascontiguousarray(
        (W * WS).reshape(njt, 128, kt, 128).transpose(0, 3, 2, 1)
    ).astype(F8)


def _pack_act_f8(aT):
    # aT [K, BL] -> [128 p, kt, BL] with [p, k, b] = aT[k*128+p, b]
    K = aT.shape[0]
    kt = K // 128
    return np.ascontiguousarray(aT.reshape(kt, 128, BL).transpose(1, 0, 2)).astype(F8)


def prepare_in_maps(x, h, c, Wi, bi, Ws, bs):
    Wi = np.asarray(Wi, np.float32)
    Ws = np.asarray(Ws, np.float32)
    wib_p = _pack_weights_bf16(Wi, NJI)
    wsb_p = _pack_weights_bf16(Ws, NJS)
    wi8_p = _pack_weights_f8(Wi, NJI)
    ws8_p = _pack_weights_f8(Ws, NJS)
    bias_comb = np.concatenate(
        [np.asarray(bi[: 5 * H], np.float32) + np.asarray(bs, np.float32),
         np.asarray(bi[5 * H :], np.float32)]
    )
    bias_pack = np.ascontiguousarray(bias_comb.reshape(NJI, 128).T).astype(np.float32)

    in_maps = []
    for i in range(N_CORES):
        s = slice(i * BL, (i + 1) * BL)
        xT = np.ascontiguousarray(np.asarray(x[s], np.float32).T)
        hT = np.ascontiguousarray(np.asarray(h[s], np.float32).T)
        in_maps.append(
            {
                "xb": xT.astype(BF16),
                "hb": hT.astype(BF16),
                "x8": _pack_act_f8(xT),
                "h8": _pack_act_f8(hT),
                "cT": np.ascontiguousarray(np.asarray(c[s], np.float32).T).astype(BF16),
                "wib": wib_p,
                "wsb": wsb_p,
                "wi8": wi8_p,
                "ws8": ws8_p,
                "bias": bias_pack,
            }
        )
    return in_maps


def run(in_maps, trace=False):
    nc = _get_nc()
    res = run_bass_kernel_spmd(nc, in_maps, core_ids=list(range(N_CORES)), trace=trace)
    out = np.empty((B, H), np.float32)
    mem = np.empty((B, H), np.float32)
    for i in range(N_CORES):
        s = slice(i * BL, (i + 1) * BL)
        out[s] = res.results[i]["outT"].T.astype(np.float32)
        mem[s] = res.results[i]["memT"].T.astype(np.float32)
    return (out, mem), res


def kernel(x, h, c, Wi, bi, Ws, bs):
    in_maps = prepare_in_maps(x, h, c, Wi, bi, Ws, bs)
    (out, mem), _ = run(in_maps, trace=False)
    return out, mem
